# revision 14
# baseline (speedup 1.0000x reference)
"""KnnXLMultiHeadsAttention on 8 TRN2 NeuronCores.

Sharding: core c -> (batch b = c//2, head-half hh = c%2, i.e. 8 of 16 heads,
a 512-wide slice of HD=1024). q/k projections are computed in full on each
core (the l2-norm spans all of HD); v/attention/output-projection work only
on the core's own head slice. Output projection partials are summed on the
host (the only cross-core reduction).

All matmuls run as float32r (same PE numerics as float32 on TRN2, 4x the
throughput at moving-dim >= 256). Scores are computed transposed
(j on partitions, i free) so softmax's sum lands in a ones-column of the
attention-value matmul and no on-chip transposes are needed.
"""
import sys
sys.path.insert(0, "/opt/trn_rl_repo")
import os

import numpy as np

B, S, E = 4, 512, 1024
H, D = 16, 64
HD = H * D
XL = 512
TOPK = 8
NJ = XL + S          # 1024
NJE = S * TOPK       # 4096
SCALE = float(D) ** -0.5

_cache = {}


def _build_nc():
    import concourse.bass as bass
    import concourse.mybir as mybir
    import concourse.tile as tile

    F32 = mybir.dt.float32
    F32R = mybir.dt.float32r
    AF = mybir.ActivationFunctionType
    ALU = mybir.AluOpType

    nc = bass.Bass()
    dram = {}
    for name, shape in [
        ("xT", (128, 8, 512)),
        ("wq", (8, 128, 8, 128)),
        ("wk", (8, 128, 8, 128)),
        ("wv", (128, 8, 512)),
        ("wo", (8, 64, 8, 128)),
        ("bq", (128, 8)),
        ("bk", (128, 8)),
        ("bv", (1, 512)),
        ("gate8", (1, 8)),
        ("kmt", (128, 4, 512)),
        ("vmav", (128, 4, 8, 65)),
        ("kxt", (4, 128, 4096)),
        ("vxav", (8, 128, 32, 65)),
        ("relm", (8, 8, 128, 512)),
    ]:
        dram[name] = nc.dram_tensor(name, list(shape), F32, kind="ExternalInput")
    o_out = nc.dram_tensor("o_out", [8, 128, 512], F32, kind="ExternalOutput")
    k_out = nc.dram_tensor("k_out", [128, 4, 512], F32, kind="ExternalOutput")
    v_out = nc.dram_tensor("v_out", [4, 128, 512], F32, kind="ExternalOutput")

    with tile.TileContext(nc) as tc, \
         nc.allow_low_precision(reason="float32r tiles hold fp32 data; PE rounds identically to its fp32 path"):
        with tc.tile_pool(name="cst", bufs=1) as cst, \
             tc.tile_pool(name="big", bufs=1) as big, \
             tc.tile_pool(name="st", bufs=2) as st, \
             tc.tile_pool(name="kxp", bufs=2) as kxp, \
             tc.tile_pool(name="vxp", bufs=1) as vxp, \
             tc.tile_pool(name="zp", bufs=2) as zp, \
             tc.tile_pool(name="ps", bufs=2, space="PSUM") as ps, \
             tc.tile_pool(name="pp", bufs=1, space="PSUM") as pp, \
             tc.tile_pool(name="pa", bufs=1, space="PSUM") as pa:

            onesf = cst.tile([128, 128], F32, tag="onesf")
            nc.any.memset(onesf[:], 1.0)
            ones = cst.tile([128, 128], F32R, tag="ones")
            nc.vector.tensor_copy(ones[:], onesf[:])

            # gate at lane 64 (sumexp rows live on partition 64)
            gt = cst.tile([65, 8], F32, tag="gt")
            nc.sync.dma_start(gt[64:65, :], dram["gate8"][:])
            gsig = cst.tile([65, 8], F32, tag="gsig")
            nc.scalar.activation(gsig[64:65, :], gt[64:65, :], AF.Sigmoid)
            gneg = cst.tile([65, 8], F32, tag="gneg")
            nc.any.tensor_scalar(gneg[64:65, :], gsig[64:65, :], -1.0, 1.0,
                                 ALU.mult, ALU.add)

            bqs = cst.tile([128, 8], F32, tag="bqs")
            nc.sync.dma_start(bqs[:], dram["bq"][:])
            bks = cst.tile([128, 8], F32, tag="bks")
            nc.sync.dma_start(bks[:], dram["bk"][:])
            bvs = cst.tile([1, 512], F32R, tag="bvs")
            nc.sync.dma_start(bvs[:], dram["bv"][:].bitcast(F32R))

            xT = big.tile([128, 8, 512], F32R, tag="xT")
            nc.sync.dma_start(xT[:], dram["xT"][:].bitcast(F32R))

            # ---------------- q/k projections + l2 norm ----------------
            qkn = {}
            for pname, wdr, bsb in (("q", dram["wq"], bqs), ("k", dram["wk"], bks)):
                raw = big.tile([128, 4, 512], F32, tag="raw", name=f"{pname}raw")
                ssp = pa.tile([1, 512], F32, tag="bc", name="ssp")
                for c in range(8):
                    wt = st.tile([128, 8, 128], F32R, tag="wt")
                    nc.sync.dma_start(wt[:], wdr[c].bitcast(F32R))
                    ppj = pp.tile([128, 512], F32, tag="pj")
                    for ec in range(8):
                        nc.tensor.matmul(ppj[:], wt[:, ec, :], xT[:, ec, :],
                                         start=(ec == 0), stop=(ec == 7))
                    sq = st.tile([128, 512], F32R, tag="sq")
                    nc.scalar.activation(sq[:], ppj[:], AF.Square,
                                         bias=bsb[:, c:c + 1])
                    nc.tensor.matmul(ssp[:], ones[:, 0:1], sq[:],
                                     start=(c == 0), stop=(c == 7),
                                     skip_group_check=True)
                    hh_c = c - 4 if c >= 4 else c  # own chunk index
                    # both halves compute all 8 chunks; own 4 are chunks
                    # [own0, own0+4) selected at runtime per core via the
                    # hostside choice of which W rows go where -- here we
                    # always keep chunks 0..3 == own hd slice (host maps
                    # the core's own slice to chunks 0..3 and the other
                    # half to 4..7).
                    if c < 4:
                        nc.any.tensor_scalar(raw[:, c, :], ppj[:],
                                             bsb[:, c:c + 1], None, ALU.add)
                nrm = st.tile([1, 512], F32, tag="nrm")
                nc.scalar.activation(nrm[:], ssp[:], AF.Sqrt)
                nc.any.tensor_scalar_max(nrm[:], nrm[:], 1e-12)
                rcp = st.tile([1, 512], F32R, tag="rcp")
                nc.vector.reciprocal(rcp[:], nrm[:])
                bcp = pa.tile([128, 512], F32, tag="bc", name="bcp")
                nc.tensor.matmul(bcp[:], ones[0:1, :], rcp[:],
                                 start=True, stop=True)
                bcs = st.tile([128, 512], F32, tag="bcast", name="bcs")
                nc.vector.tensor_copy(bcs[:], bcp[:])
                tn = big.tile([128, 4, 512], F32R, tag=f"{pname}n")
                for c in range(4):
                    nc.vector.tensor_mul(tn[:, c, :], raw[:, c, :], bcs[:])
                qkn[pname] = tn
            qn, kn = qkn["q"], qkn["k"]

            # ---------------- v projection (natural layout) ----------------
            vnav = big.tile([128, 4, 8, 65], F32R, tag="vnav")
            for it in range(4):
                pv = pp.tile([128, 512], F32, tag="pj")
                nc.tensor.matmul(pv[:], ones[0:1, :], bvs[:],
                                 start=True, stop=False)
                for ec in range(8):
                    wvt = st.tile([128, 512], F32R, tag="wvt")
                    nc.sync.dma_start(wvt[:], dram["wv"][:, ec, :].bitcast(F32R))
                    nc.tensor.matmul(pv[:], xT[:, ec, it * 128:(it + 1) * 128],
                                     wvt[:], start=False, stop=(ec == 7))
                vcp = st.tile([128, 512], F32, tag="vcp")
                nc.vector.tensor_copy(vcp[:], pv[:])
                nc.sync.dma_start(v_out[it], vcp[:])
                for h in range(8):
                    nc.vector.tensor_copy(vnav[:, it, h, 0:64],
                                          pv[:, h * 64:(h + 1) * 64])
            nc.vector.tensor_copy(
                vnav[:, :, :, 64:65].rearrange("p a b c -> p (a b c)"),
                onesf[:, 0:32])

            kmt = big.tile([128, 4, 512], F32R, tag="kmt")
            nc.sync.dma_start(kmt[:], dram["kmt"][:].bitcast(F32R))
            vmav = big.tile([128, 4, 8, 65], F32R, tag="vmav")
            nc.sync.dma_start(vmav[:], dram["vmav"][:].bitcast(F32R))

            nc.sync.dma_start(k_out[:], kn[:].bitcast(F32))

            # ---------------- attention per head ----------------
            fusedT = big.tile([64, 8, 512], F32R, tag="fusedT")
            for h in range(8):
                par = (h % 2) * 64
                ch = h // 2
                qh = qn[par:par + 64, ch, :]
                if h % 2 == 0:
                    kxh = [None, None]
                    for half in range(2):
                        kxh[half] = kxp.tile([128, 2048], F32R,
                                             tag=f"kx{half}", name=f"kx{half}")
                        nc.sync.dma_start(
                            kxh[half][:],
                            dram["kxt"][h // 2, :,
                                        half * 2048:(half + 1) * 2048]
                            .bitcast(F32R))
                vx = vxp.tile([128, 32, 65], F32R, tag="vx")
                nc.sync.dma_start(vx[:], dram["vxav"][h].bitcast(F32R))

                # ---- main (xl + current) attention, 8 j-chunks of 128 ----
                avm = pa.tile([65, 512], F32, tag="avm")
                for jc2 in range(4):
                    sc = ps.tile([128, 1024], F32, tag="sc")
                    for k in range(2):
                        jc = jc2 * 2 + k
                        if jc < 4:
                            kl = kmt[par:par + 64, ch, jc * 128:(jc + 1) * 128]
                        else:
                            kl = kn[par:par + 64, ch,
                                    (jc - 4) * 128:(jc - 3) * 128]
                        nc.tensor.matmul(sc[:, k * 512:(k + 1) * 512], kl, qh,
                                         start=True, stop=True)
                    rt = st.tile([128, 1024], F32, tag="rt")
                    nc.sync.dma_start(
                        rt[:].rearrange("p (a x) -> p a x", a=2),
                        dram["relm"][h, jc2 * 2:jc2 * 2 + 2].rearrange(
                            "a p x -> p a x"))
                    nc.vector.tensor_add(sc[:], sc[:], rt[:])
                    z = zp.tile([128, 1024], F32R, tag="z")
                    nc.scalar.activation(z[:], sc[:], AF.Exp, scale=SCALE)
                    for k in range(2):
                        jc = jc2 * 2 + k
                        if jc < 4:
                            vl = vmav[:, jc, h, :]
                        else:
                            vl = vnav[:, jc - 4, h, :]
                        nc.tensor.matmul(avm[:], vl,
                                         z[:, k * 512:(k + 1) * 512],
                                         start=(jc == 0), stop=(jc == 7),
                                         skip_group_check=True)

                # ---- external knn attention, 32 j-chunks of 128 ----
                ave = pa.tile([65, 512], F32, tag="ave")
                for jc2 in range(16):
                    sc = ps.tile([128, 1024], F32, tag="sc")
                    for k in range(2):
                        jj = jc2 * 2 + k
                        nc.tensor.matmul(
                            sc[:, k * 512:(k + 1) * 512],
                            kxh[jj // 16][par:par + 64,
                                          (jj % 16) * 128:(jj % 16 + 1) * 128],
                            qh, start=True, stop=True)
                    z = zp.tile([128, 1024], F32R, tag="z")
                    nc.scalar.activation(z[:], sc[:], AF.Exp, scale=SCALE)
                    for k in range(2):
                        jj = jc2 * 2 + k
                        nc.tensor.matmul(ave[:], vx[:, jj, :],
                                         z[:, k * 512:(k + 1) * 512],
                                         start=(jj == 0), stop=(jj == 31),
                                         skip_group_check=True)

                # ---- gating / softmax normalization ----
                rcm = st.tile([65, 512], F32R, tag="rcm")
                nc.vector.reciprocal(rcm[64:65, :], avm[64:65, :])
                rce = st.tile([65, 512], F32R, tag="rce")
                nc.vector.reciprocal(rce[64:65, :], ave[64:65, :])
                rgm = st.tile([65, 512], F32R, tag="rgm")
                nc.any.tensor_scalar(rgm[64:65, :], rcm[64:65, :],
                                     gsig[64:65, h:h + 1], None, ALU.mult)
                rge = st.tile([65, 512], F32R, tag="rge")
                nc.any.tensor_scalar(rge[64:65, :], rce[64:65, :],
                                     gneg[64:65, h:h + 1], None, ALU.mult)
                onr = ones[64:65, 0:64]
                bcg = pa.tile([64, 512], F32, tag="bc", name="bcg")
                nc.tensor.matmul(bcg[:], onr, rgm[64:65, :],
                                 start=True, stop=True, skip_group_check=True)
                bgs1 = st.tile([64, 512], F32, tag="bcast", name="bgs1")
                nc.vector.tensor_copy(bgs1[:], bcg[:])
                bcg2 = pa.tile([64, 512], F32, tag="bc", name="bcg2")
                nc.tensor.matmul(bcg2[:], onr, rge[64:65, :],
                                 start=True, stop=True, skip_group_check=True)
                bgs2 = st.tile([64, 512], F32, tag="bcast", name="bgs2")
                nc.vector.tensor_copy(bgs2[:], bcg2[:])
                t1 = vxp.tile([64, 512], F32, tag="t1")
                nc.vector.tensor_mul(t1[:], avm[0:64, :], bgs1[:])
                t2 = vxp.tile([64, 512], F32, tag="t2")
                nc.vector.tensor_mul(t2[:], ave[0:64, :], bgs2[:])
                nc.vector.tensor_add(fusedT[:, h, :], t1[:], t2[:])

            # ---------------- output projection ----------------
            for et in range(8):
                wot = st.tile([64, 8, 128], F32R, tag="wot")
                nc.sync.dma_start(wot[:], dram["wo"][et].bitcast(F32R))
                po = pp.tile([128, 512], F32, tag="pj")
                for h in range(8):
                    nc.tensor.matmul(po[:],
                                     wot[:, h, :],
                                     fusedT[:, h, :],
                                     start=(h == 0), stop=(h == 7))
                ocp = st.tile([128, 512], F32, tag="ocp")
                nc.vector.tensor_copy(ocp[:], po[:])
                nc.sync.dma_start(o_out[et], ocp[:])

    from sync_fix import fix_sync_waits
    fix_sync_waits(nc)
    return nc


def _prep_inputs(input, rel_pos, xl_memory, kv_external, Wq, bq, Wk, bk,
                 Wv, bv, Wo, bo, gate):
    """Build the 8 per-core input maps. Core c: b=c//2, hh=c%2."""
    f32 = np.float32
    ac = np.ascontiguousarray
    maps = []
    # masked+transposed rel_pos per head: [h, j, i]
    ii = np.arange(S)[None, :]
    jj = np.arange(NJ)[:, None]
    mask = (jj > ii + XL)  # masked where j >= i + 513
    for c in range(8):
        b, hh = c // 2, c % 2
        sl0 = hh * 512
        sl = slice(sl0, sl0 + 512)
        m = {}
        # xT[p, ec, i] = input[b, i, ec*128+p]
        m["xT"] = ac(input[b].T.reshape(8, 128, S).transpose(1, 0, 2)).astype(f32)

        # wq[c2, p, ec, mcol]: own half's rows mapped to chunks 0..3
        def wmap(W):
            # chunk order: own half first (chunks 0..3), other half 4..7
            order = list(range(hh * 4, hh * 4 + 4)) + \
                    list(range((1 - hh) * 4, (1 - hh) * 4 + 4))
            out = np.empty((8, 128, 8, 128), f32)
            WT = W.T  # (E, HD)
            for ci, corig in enumerate(order):
                blk = WT[:, corig * 128:(corig + 1) * 128]  # (E, 128)
                out[ci] = blk.reshape(8, 128, 128).transpose(1, 0, 2)
            return ac(out)
        m["wq"] = wmap(Wq)
        m["wk"] = wmap(Wk)

        def bmap(bvec):
            order = list(range(hh * 4, hh * 4 + 4)) + \
                    list(range((1 - hh) * 4, (1 - hh) * 4 + 4))
            out = np.empty((128, 8), f32)
            for ci, corig in enumerate(order):
                out[:, ci] = bvec[corig * 128:(corig + 1) * 128]
            return ac(out)
        m["bq"] = bmap(np.asarray(bq))
        m["bk"] = bmap(np.asarray(bk))

        # wv[p, ec, x] = Wv[sl0+x, ec*128+p]
        m["wv"] = ac(Wv[sl, :].T.reshape(8, 128, 512).transpose(1, 0, 2)).astype(f32)
        # wo[et, d, h, m] = Wo[et*128+m, sl0+h*64+d]
        woT = Wo[:, sl].T.reshape(8, 64, 8, 128)     # (h, d, et, m)
        m["wo"] = ac(woT.transpose(2, 1, 0, 3)).astype(f32)
        m["bv"] = ac(np.asarray(bv)[sl][None, :]).astype(f32)
        m["gate8"] = ac(np.asarray(gate)[hh * 8:hh * 8 + 8, 0, 0][None, :]).astype(f32)

        # kmt[p, cc, x] = xl_memory[b, x, 0, sl0+cc*128+p]
        km = xl_memory[b, :, 0, sl]          # (XL, 512)
        m["kmt"] = ac(km.T.reshape(4, 128, XL).transpose(1, 0, 2)).astype(f32)
        # vmav[p, jc, h, d(+1)] = xl_memory[b, jc*128+p, 1, sl0+h*64+d]
        vm = xl_memory[b, :, 1, sl].reshape(4, 128, 8, 64)
        vmav = np.ones((128, 4, 8, 65), f32)
        vmav[:, :, :, :64] = vm.transpose(1, 0, 2, 3)
        m["vmav"] = ac(vmav)

        # kxt[pr, p, j'] = kv_ext k at head 2pr + p//64, d=p%64, j'=s*8+t
        ke = kv_external[b, :, :, 0, sl].reshape(NJE, 8, 64)  # (j', h, d)
        kxt = np.empty((4, 128, NJE), f32)
        for pr in range(4):
            kxt[pr, 0:64] = ke[:, 2 * pr, :].T
            kxt[pr, 64:128] = ke[:, 2 * pr + 1, :].T
        m["kxt"] = ac(kxt)
        # vxav[h, p, cc, d] = kv_ext v at j' = cc*128+p
        ve = kv_external[b, :, :, 1, sl].reshape(32, 128, 8, 64)
        vxav = np.ones((8, 128, 32, 65), f32)
        vxav[:, :, :, :64] = ve.transpose(2, 1, 0, 3)
        m["vxav"] = ac(vxav)

        # relm[h, jc, p, i] = masked relT
        rel = np.asarray(rel_pos)[0, hh * 8:hh * 8 + 8]      # (8, S, NJ)
        relT = np.where(mask[None], np.float32(-1e9),
                        rel.transpose(0, 2, 1))               # (8, NJ, S)
        m["relm"] = ac(relT.reshape(8, 8, 128, S)).astype(f32)
        maps.append({k: np.ascontiguousarray(v, dtype=np.float32)
                     for k, v in m.items()})
    return maps


def kernel(input, rel_pos, xl_memory, kv_external, Wq, bq, Wk, bk, Wv, bv,
           Wo, bo, gate):
    from concourse.bass_utils import run_bass_kernel_spmd

    input = np.asarray(input)
    if "nc" not in _cache:
        _cache["nc"] = _build_nc()
    nc = _cache["nc"]

    maps = _prep_inputs(input, np.asarray(rel_pos), np.asarray(xl_memory),
                        np.asarray(kv_external), np.asarray(Wq),
                        np.asarray(bq), np.asarray(Wk), np.asarray(bk),
                        np.asarray(Wv), np.asarray(bv), np.asarray(Wo),
                        np.asarray(bo), np.asarray(gate))

    res = run_bass_kernel_spmd(nc, maps, core_ids=list(range(8)))
    _cache["last_res"] = res

    out = np.zeros((B, S, E), np.float32)
    current_kv = np.zeros((B, S, 2, HD), np.float32)
    for c in range(8):
        b, hh = c // 2, c % 2
        sl = slice(hh * 512, hh * 512 + 512)
        r = res.results[c]
        # o_out[et, p, i]: partial outT -> out[b, i, et*128+p]
        oT = r["o_out"].reshape(E, S)          # (e, i)
        out[b] += oT.T
        # k_out[p, cc, x] -> normalized k at hd = sl0+cc*128+p, pos x
        kT = r["k_out"].transpose(1, 0, 2).reshape(512, S)   # (hd_local, i)
        current_kv[b, :, 0, sl] = kT.T
        current_kv[b, :, 1, sl] = r["v_out"].reshape(S, 512)
    out += np.asarray(bo)[None, None, :]
    return out, current_kv


# revision 15
# speedup vs baseline: 1.8078x; 1.8078x over previous
"""KnnXLMultiHeadsAttention on 8 TRN2 NeuronCores.

Sharding: core c -> (batch b = c//2, head-half hh = c%2, i.e. 8 of 16 heads,
a 512-wide slice of HD=1024). q/k projections are computed in full on each
core (the l2-norm spans all of HD); v/attention/output-projection work only
on the core's own head slice. Output projection partials are summed on the
host (the only cross-core reduction).

All matmuls run as float32r (same PE numerics as float32 on TRN2, 4x the
throughput at moving-dim >= 256). Scores are computed transposed
(j on partitions, i free) so softmax's sum lands in a ones-column of the
attention-value matmul and no on-chip transposes are needed.
"""
import sys
sys.path.insert(0, "/opt/trn_rl_repo")
import os

import numpy as np

B, S, E = 4, 512, 1024
H, D = 16, 64
HD = H * D
XL = 512
TOPK = 8
NJ = XL + S          # 1024
NJE = S * TOPK       # 4096
SCALE = float(D) ** -0.5

_cache = {}


def _fix_sync_waits(nc, mybir):
    """walrus on this toolchain rejects >1 semaphore wait per instruction;
    move excess waits onto preceding same-engine NoOps (queue order keeps
    semantics identical)."""
    counter = [0]

    def mk_nop(engine, waits):
        counter[0] += 1
        nop = mybir.InstNoOp(name=f"syncfix-nop-{counter[0]}", ins=[], outs=[])
        nop.engine = engine
        nop.sync_info = mybir.SyncInfo(on_wait=list(waits), on_update=[])
        return nop

    for f in nc.m.functions:
        for blk in f.blocks:
            out = []
            for inst in blk.instructions:
                si = getattr(inst, "sync_info", None)
                waits = list(si.on_wait) if si and si.on_wait else []
                if len(waits) > 1 and inst.engine != mybir.EngineType.Unassigned:
                    keep, extra = waits[:1], waits[1:]
                    while extra:
                        chunk, extra = extra[:1], extra[1:]
                        out.append(mk_nop(inst.engine, chunk))
                    inst.sync_info = mybir.SyncInfo(
                        on_wait=keep, on_update=list(si.on_update or []))
                out.append(inst)
            blk.instructions[:] = out
    return nc




def _build_nc():
    import concourse.bass as bass
    import concourse.mybir as mybir
    import concourse.tile as tile

    F32 = mybir.dt.float32
    F32R = mybir.dt.float32r
    AF = mybir.ActivationFunctionType
    ALU = mybir.AluOpType

    nc = bass.Bass()
    dram = {}
    for name, shape in [
        ("xT", (128, 8, 512)),
        ("wq", (8, 128, 8, 128)),
        ("wk", (8, 128, 8, 128)),
        ("wv", (128, 8, 512)),
        ("wo", (8, 64, 8, 128)),
        ("bq", (128, 8)),
        ("bk", (128, 8)),
        ("bv", (1, 512)),
        ("gate8", (1, 8)),
        ("kmt", (128, 4, 512)),
        ("vmav", (128, 4, 8, 65)),
        ("kxt", (4, 128, 4096)),
        ("vxav", (8, 128, 32, 65)),
        ("relm", (8, 8, 128, 512)),
    ]:
        dram[name] = nc.dram_tensor(name, list(shape), F32, kind="ExternalInput")
    o_out = nc.dram_tensor("o_out", [8, 128, 512], F32, kind="ExternalOutput")
    k_out = nc.dram_tensor("k_out", [128, 4, 512], F32, kind="ExternalOutput")
    v_out = nc.dram_tensor("v_out", [4, 128, 512], F32, kind="ExternalOutput")

    with tile.TileContext(nc) as tc, \
         nc.allow_low_precision(reason="float32r tiles hold fp32 data; PE rounds identically to its fp32 path"):
        with tc.tile_pool(name="cst", bufs=1) as cst, \
             tc.tile_pool(name="big", bufs=1) as big, \
             tc.tile_pool(name="st", bufs=2) as st, \
             tc.tile_pool(name="kxp", bufs=2) as kxp, \
             tc.tile_pool(name="vxp", bufs=1) as vxp, \
             tc.tile_pool(name="zp", bufs=2) as zp, \
             tc.tile_pool(name="ps", bufs=2, space="PSUM") as ps, \
             tc.tile_pool(name="pp", bufs=1, space="PSUM") as pp, \
             tc.tile_pool(name="pa", bufs=1, space="PSUM") as pa:

            onesf = cst.tile([128, 128], F32, tag="onesf")
            nc.any.memset(onesf[:], 1.0)
            ones = cst.tile([128, 128], F32R, tag="ones")
            nc.vector.tensor_copy(ones[:], onesf[:])

            # gate at lane 64 (sumexp rows live on partition 64)
            gt = cst.tile([65, 8], F32, tag="gt")
            nc.sync.dma_start(gt[64:65, :], dram["gate8"][:])
            gsig = cst.tile([65, 8], F32, tag="gsig")
            nc.scalar.activation(gsig[64:65, :], gt[64:65, :], AF.Sigmoid)
            gneg = cst.tile([65, 8], F32, tag="gneg")
            nc.any.tensor_scalar(gneg[64:65, :], gsig[64:65, :], -1.0, 1.0,
                                 ALU.mult, ALU.add)

            bqs = cst.tile([128, 8], F32, tag="bqs")
            nc.sync.dma_start(bqs[:], dram["bq"][:])
            bks = cst.tile([128, 8], F32, tag="bks")
            nc.sync.dma_start(bks[:], dram["bk"][:])
            bvs = cst.tile([1, 512], F32R, tag="bvs")
            nc.sync.dma_start(bvs[:], dram["bv"][:].bitcast(F32R))

            xT = big.tile([128, 8, 512], F32R, tag="xT")
            nc.sync.dma_start(xT[:], dram["xT"][:].bitcast(F32R))

            # ---------------- q/k projections + l2 norm ----------------
            qkn = {}
            for pname, wdr, bsb in (("q", dram["wq"], bqs), ("k", dram["wk"], bks)):
                raw = big.tile([128, 4, 512], F32, tag="raw", name=f"{pname}raw")
                ssp = pa.tile([1, 512], F32, tag="bc", name="ssp")
                for c in range(8):
                    wt = st.tile([128, 8, 128], F32R, tag="wt")
                    nc.sync.dma_start(wt[:], wdr[c].bitcast(F32R))
                    ppj = pp.tile([128, 512], F32, tag="pj")
                    for ec in range(8):
                        nc.tensor.matmul(ppj[:], wt[:, ec, :], xT[:, ec, :],
                                         start=(ec == 0), stop=(ec == 7))
                    sq = st.tile([128, 512], F32R, tag="sq")
                    nc.scalar.activation(sq[:], ppj[:], AF.Square,
                                         bias=bsb[:, c:c + 1])
                    nc.tensor.matmul(ssp[:], ones[:, 0:1], sq[:],
                                     start=(c == 0), stop=(c == 7),
                                     skip_group_check=True)
                    hh_c = c - 4 if c >= 4 else c  # own chunk index
                    # both halves compute all 8 chunks; own 4 are chunks
                    # [own0, own0+4) selected at runtime per core via the
                    # hostside choice of which W rows go where -- here we
                    # always keep chunks 0..3 == own hd slice (host maps
                    # the core's own slice to chunks 0..3 and the other
                    # half to 4..7).
                    if c < 4:
                        nc.any.tensor_scalar(raw[:, c, :], ppj[:],
                                             bsb[:, c:c + 1], None, ALU.add)
                nrm = st.tile([1, 512], F32, tag="nrm")
                nc.scalar.activation(nrm[:], ssp[:], AF.Sqrt)
                nc.any.tensor_scalar_max(nrm[:], nrm[:], 1e-12)
                rcp = st.tile([1, 512], F32R, tag="rcp")
                nc.vector.reciprocal(rcp[:], nrm[:])
                bcp = pa.tile([128, 512], F32, tag="bc", name="bcp")
                nc.tensor.matmul(bcp[:], ones[0:1, :], rcp[:],
                                 start=True, stop=True)
                bcs = st.tile([128, 512], F32, tag="bcast", name="bcs")
                nc.vector.tensor_copy(bcs[:], bcp[:])
                tn = big.tile([128, 4, 512], F32R, tag=f"{pname}n")
                for c in range(4):
                    nc.vector.tensor_mul(tn[:, c, :], raw[:, c, :], bcs[:])
                qkn[pname] = tn
            qn, kn = qkn["q"], qkn["k"]

            # ---------------- v projection (natural layout) ----------------
            vnav = big.tile([128, 4, 8, 65], F32R, tag="vnav")
            for it in range(4):
                pv = pp.tile([128, 512], F32, tag="pj")
                nc.tensor.matmul(pv[:], ones[0:1, :], bvs[:],
                                 start=True, stop=False)
                for ec in range(8):
                    wvt = st.tile([128, 512], F32R, tag="wvt")
                    nc.sync.dma_start(wvt[:], dram["wv"][:, ec, :].bitcast(F32R))
                    nc.tensor.matmul(pv[:], xT[:, ec, it * 128:(it + 1) * 128],
                                     wvt[:], start=False, stop=(ec == 7))
                vcp = st.tile([128, 512], F32, tag="vcp")
                nc.vector.tensor_copy(vcp[:], pv[:])
                nc.sync.dma_start(v_out[it], vcp[:])
                for h in range(8):
                    nc.vector.tensor_copy(vnav[:, it, h, 0:64],
                                          pv[:, h * 64:(h + 1) * 64])
            nc.vector.tensor_copy(
                vnav[:, :, :, 64:65].rearrange("p a b c -> p (a b c)"),
                onesf[:, 0:32])

            kmt = big.tile([128, 4, 512], F32R, tag="kmt")
            nc.sync.dma_start(kmt[:], dram["kmt"][:].bitcast(F32R))
            vmav = big.tile([128, 4, 8, 65], F32R, tag="vmav")
            nc.sync.dma_start(vmav[:], dram["vmav"][:].bitcast(F32R))

            nc.sync.dma_start(k_out[:], kn[:].bitcast(F32))

            # ---------------- attention per head ----------------
            fusedT = big.tile([64, 8, 512], F32R, tag="fusedT")
            for h in range(8):
                par = (h % 2) * 64
                ch = h // 2
                qh = qn[par:par + 64, ch, :]
                if h % 2 == 0:
                    kxh = [None, None]
                    for half in range(2):
                        kxh[half] = kxp.tile([128, 2048], F32R,
                                             tag=f"kx{half}", name=f"kx{half}")
                        nc.sync.dma_start(
                            kxh[half][:],
                            dram["kxt"][h // 2, :,
                                        half * 2048:(half + 1) * 2048]
                            .bitcast(F32R))
                vx = vxp.tile([128, 32, 65], F32R, tag="vx")
                nc.sync.dma_start(vx[:], dram["vxav"][h].bitcast(F32R))

                # ---- main (xl + current) attention, 8 j-chunks of 128 ----
                avm = pa.tile([65, 512], F32, tag="avm")
                for jc2 in range(4):
                    sc = ps.tile([128, 1024], F32, tag="sc")
                    for k in range(2):
                        jc = jc2 * 2 + k
                        if jc < 4:
                            kl = kmt[par:par + 64, ch, jc * 128:(jc + 1) * 128]
                        else:
                            kl = kn[par:par + 64, ch,
                                    (jc - 4) * 128:(jc - 3) * 128]
                        nc.tensor.matmul(sc[:, k * 512:(k + 1) * 512], kl, qh,
                                         start=True, stop=True)
                    rt = st.tile([128, 1024], F32, tag="rt")
                    nc.sync.dma_start(
                        rt[:].rearrange("p (a x) -> p a x", a=2),
                        dram["relm"][h, jc2 * 2:jc2 * 2 + 2].rearrange(
                            "a p x -> p a x"))
                    nc.vector.tensor_add(sc[:], sc[:], rt[:])
                    z = zp.tile([128, 1024], F32R, tag="z")
                    nc.scalar.activation(z[:], sc[:], AF.Exp, scale=SCALE)
                    for k in range(2):
                        jc = jc2 * 2 + k
                        if jc < 4:
                            vl = vmav[:, jc, h, :]
                        else:
                            vl = vnav[:, jc - 4, h, :]
                        nc.tensor.matmul(avm[:], vl,
                                         z[:, k * 512:(k + 1) * 512],
                                         start=(jc == 0), stop=(jc == 7),
                                         skip_group_check=True)

                # ---- external knn attention, 32 j-chunks of 128 ----
                ave = pa.tile([65, 512], F32, tag="ave")
                for jc2 in range(16):
                    sc = ps.tile([128, 1024], F32, tag="sc")
                    for k in range(2):
                        jj = jc2 * 2 + k
                        nc.tensor.matmul(
                            sc[:, k * 512:(k + 1) * 512],
                            kxh[jj // 16][par:par + 64,
                                          (jj % 16) * 128:(jj % 16 + 1) * 128],
                            qh, start=True, stop=True)
                    z = zp.tile([128, 1024], F32R, tag="z")
                    nc.scalar.activation(z[:], sc[:], AF.Exp, scale=SCALE)
                    for k in range(2):
                        jj = jc2 * 2 + k
                        nc.tensor.matmul(ave[:], vx[:, jj, :],
                                         z[:, k * 512:(k + 1) * 512],
                                         start=(jj == 0), stop=(jj == 31),
                                         skip_group_check=True)

                # ---- gating / softmax normalization ----
                rcm = st.tile([65, 512], F32R, tag="rcm")
                nc.vector.reciprocal(rcm[64:65, :], avm[64:65, :])
                rce = st.tile([65, 512], F32R, tag="rce")
                nc.vector.reciprocal(rce[64:65, :], ave[64:65, :])
                rgm = st.tile([65, 512], F32R, tag="rgm")
                nc.any.tensor_scalar(rgm[64:65, :], rcm[64:65, :],
                                     gsig[64:65, h:h + 1], None, ALU.mult)
                rge = st.tile([65, 512], F32R, tag="rge")
                nc.any.tensor_scalar(rge[64:65, :], rce[64:65, :],
                                     gneg[64:65, h:h + 1], None, ALU.mult)
                onr = ones[64:65, 0:64]
                bcg = pa.tile([64, 512], F32, tag="bc", name="bcg")
                nc.tensor.matmul(bcg[:], onr, rgm[64:65, :],
                                 start=True, stop=True, skip_group_check=True)
                bgs1 = st.tile([64, 512], F32, tag="bcast", name="bgs1")
                nc.vector.tensor_copy(bgs1[:], bcg[:])
                bcg2 = pa.tile([64, 512], F32, tag="bc", name="bcg2")
                nc.tensor.matmul(bcg2[:], onr, rge[64:65, :],
                                 start=True, stop=True, skip_group_check=True)
                bgs2 = st.tile([64, 512], F32, tag="bcast", name="bgs2")
                nc.vector.tensor_copy(bgs2[:], bcg2[:])
                t1 = vxp.tile([64, 512], F32, tag="t1")
                nc.vector.tensor_mul(t1[:], avm[0:64, :], bgs1[:])
                t2 = vxp.tile([64, 512], F32, tag="t2")
                nc.vector.tensor_mul(t2[:], ave[0:64, :], bgs2[:])
                nc.vector.tensor_add(fusedT[:, h, :], t1[:], t2[:])

            # ---------------- output projection ----------------
            for et in range(8):
                wot = st.tile([64, 8, 128], F32R, tag="wot")
                nc.sync.dma_start(wot[:], dram["wo"][et].bitcast(F32R))
                po = pp.tile([128, 512], F32, tag="pj")
                for h in range(8):
                    nc.tensor.matmul(po[:],
                                     wot[:, h, :],
                                     fusedT[:, h, :],
                                     start=(h == 0), stop=(h == 7))
                ocp = st.tile([128, 512], F32, tag="ocp")
                nc.vector.tensor_copy(ocp[:], po[:])
                nc.sync.dma_start(o_out[et], ocp[:])

    _fix_sync_waits(nc, mybir)
    return nc


def _prep_inputs(input, rel_pos, xl_memory, kv_external, Wq, bq, Wk, bk,
                 Wv, bv, Wo, bo, gate):
    """Build the 8 per-core input maps. Core c: b=c//2, hh=c%2."""
    f32 = np.float32
    ac = np.ascontiguousarray
    maps = []
    # masked+transposed rel_pos per head: [h, j, i]
    ii = np.arange(S)[None, :]
    jj = np.arange(NJ)[:, None]
    mask = (jj > ii + XL)  # masked where j >= i + 513
    for c in range(8):
        b, hh = c // 2, c % 2
        sl0 = hh * 512
        sl = slice(sl0, sl0 + 512)
        m = {}
        # xT[p, ec, i] = input[b, i, ec*128+p]
        m["xT"] = ac(input[b].T.reshape(8, 128, S).transpose(1, 0, 2)).astype(f32)

        # wq[c2, p, ec, mcol]: own half's rows mapped to chunks 0..3
        def wmap(W):
            # chunk order: own half first (chunks 0..3), other half 4..7
            order = list(range(hh * 4, hh * 4 + 4)) + \
                    list(range((1 - hh) * 4, (1 - hh) * 4 + 4))
            out = np.empty((8, 128, 8, 128), f32)
            WT = W.T  # (E, HD)
            for ci, corig in enumerate(order):
                blk = WT[:, corig * 128:(corig + 1) * 128]  # (E, 128)
                out[ci] = blk.reshape(8, 128, 128).transpose(1, 0, 2)
            return ac(out)
        m["wq"] = wmap(Wq)
        m["wk"] = wmap(Wk)

        def bmap(bvec):
            order = list(range(hh * 4, hh * 4 + 4)) + \
                    list(range((1 - hh) * 4, (1 - hh) * 4 + 4))
            out = np.empty((128, 8), f32)
            for ci, corig in enumerate(order):
                out[:, ci] = bvec[corig * 128:(corig + 1) * 128]
            return ac(out)
        m["bq"] = bmap(np.asarray(bq))
        m["bk"] = bmap(np.asarray(bk))

        # wv[p, ec, x] = Wv[sl0+x, ec*128+p]
        m["wv"] = ac(Wv[sl, :].T.reshape(8, 128, 512).transpose(1, 0, 2)).astype(f32)
        # wo[et, d, h, m] = Wo[et*128+m, sl0+h*64+d]
        woT = Wo[:, sl].T.reshape(8, 64, 8, 128)     # (h, d, et, m)
        m["wo"] = ac(woT.transpose(2, 1, 0, 3)).astype(f32)
        m["bv"] = ac(np.asarray(bv)[sl][None, :]).astype(f32)
        m["gate8"] = ac(np.asarray(gate)[hh * 8:hh * 8 + 8, 0, 0][None, :]).astype(f32)

        # kmt[p, cc, x] = xl_memory[b, x, 0, sl0+cc*128+p]
        km = xl_memory[b, :, 0, sl]          # (XL, 512)
        m["kmt"] = ac(km.T.reshape(4, 128, XL).transpose(1, 0, 2)).astype(f32)
        # vmav[p, jc, h, d(+1)] = xl_memory[b, jc*128+p, 1, sl0+h*64+d]
        vm = xl_memory[b, :, 1, sl].reshape(4, 128, 8, 64)
        vmav = np.ones((128, 4, 8, 65), f32)
        vmav[:, :, :, :64] = vm.transpose(1, 0, 2, 3)
        m["vmav"] = ac(vmav)

        # kxt[pr, p, j'] = kv_ext k at head 2pr + p//64, d=p%64, j'=s*8+t
        ke = kv_external[b, :, :, 0, sl].reshape(NJE, 8, 64)  # (j', h, d)
        kxt = np.empty((4, 128, NJE), f32)
        for pr in range(4):
            kxt[pr, 0:64] = ke[:, 2 * pr, :].T
            kxt[pr, 64:128] = ke[:, 2 * pr + 1, :].T
        m["kxt"] = ac(kxt)
        # vxav[h, p, cc, d] = kv_ext v at j' = cc*128+p
        ve = kv_external[b, :, :, 1, sl].reshape(32, 128, 8, 64)
        vxav = np.ones((8, 128, 32, 65), f32)
        vxav[:, :, :, :64] = ve.transpose(2, 1, 0, 3)
        m["vxav"] = ac(vxav)

        # relm[h, jc, p, i] = masked relT
        rel = np.asarray(rel_pos)[0, hh * 8:hh * 8 + 8]      # (8, S, NJ)
        relT = np.where(mask[None], np.float32(-1e9),
                        rel.transpose(0, 2, 1))               # (8, NJ, S)
        m["relm"] = ac(relT.reshape(8, 8, 128, S)).astype(f32)
        maps.append({k: np.ascontiguousarray(v, dtype=np.float32)
                     for k, v in m.items()})
    return maps


def kernel(input, rel_pos, xl_memory, kv_external, Wq, bq, Wk, bk, Wv, bv,
           Wo, bo, gate):
    from concourse.bass_utils import run_bass_kernel_spmd

    input = np.asarray(input)
    if "nc" not in _cache:
        _cache["nc"] = _build_nc()
    nc = _cache["nc"]

    maps = _prep_inputs(input, np.asarray(rel_pos), np.asarray(xl_memory),
                        np.asarray(kv_external), np.asarray(Wq),
                        np.asarray(bq), np.asarray(Wk), np.asarray(bk),
                        np.asarray(Wv), np.asarray(bv), np.asarray(Wo),
                        np.asarray(bo), np.asarray(gate))

    res = run_bass_kernel_spmd(nc, maps, core_ids=list(range(8)))
    _cache["last_res"] = res

    out = np.zeros((B, S, E), np.float32)
    current_kv = np.zeros((B, S, 2, HD), np.float32)
    for c in range(8):
        b, hh = c // 2, c % 2
        sl = slice(hh * 512, hh * 512 + 512)
        r = res.results[c]
        # o_out[et, p, i]: partial outT -> out[b, i, et*128+p]
        oT = r["o_out"].reshape(E, S)          # (e, i)
        out[b] += oT.T
        # k_out[p, cc, x] -> normalized k at hd = sl0+cc*128+p, pos x
        kT = r["k_out"].transpose(1, 0, 2).reshape(512, S)   # (hd_local, i)
        current_kv[b, :, 0, sl] = kT.T
        current_kv[b, :, 1, sl] = r["v_out"].reshape(S, 512)
    out += np.asarray(bo)[None, None, :]
    return out, current_kv
